# revision 52
# baseline (speedup 1.0000x reference)
"""Trainium2 Bass kernel for nn_Model_11888469475921 (dense_cnn).

Computation (per image, 1024 images total):
  proj = conv3x3(x, weight) + bias          # (64, 32, 32), padding 1
  act  = selu(proj)
  pooled = sqrt(act[...,0::2]^2 + act[...,1::2]^2)   # LPPool1d(p=2,k=2) along W
  gate = sigmoid(mean_{H,W}(x) @ scale_proj + scale_bias)
  out  = mean_{H,Wp}(pooled) * gate         # (64,)

Strategy: pure data parallel over 8 NeuronCores (128 images each).

Host prep (pure layout, no FLOPs): x is cast to bf16 and laid out as (a)
xb_host -- per-chunk 6-row "ki=1 base" blocks, row (r,c) = padded
x[c, i'-1, j-1] over (i':34, p:16, j:34) -- and (b) xpad_host, the padded
(c,i)-partitioned x for the gate's channel sums.  This removes every
SWDGE cast-DMA from the Pool engine (descriptor gen used to cost it
~12us) and shrinks the startup fill chain.

Conv: FULL-K im2col, K=57 = 3 kj-blocks x (ki,r,c rows) + ones row; rows
carry PAD slack each side.  Per chunk: ONE DRAM DMA loads the 6 ki=1
rows at full 34-i-row width; ki=0/2 rows are +-PAD free-shifted copies
(the ki taps are PAD-shifted windows of the same base, like the kj=1/2
blocks are 1/2-elem shifted copies) -- 6 HWDGE DMAs/chunk vs 21 for
per-row transposes.  One K=57 matmul per 512-pixel parity group (matmul
cost is output rows only; K depth and LdWeights are free in this cost
model).  Conv bias rides the kj=1 ones row only.

Elementwise: PSUM tiles hold TWO pairs [128, 2048] to amortize per-op init.
Per tile:
  e = Exp(alpha * a~)            (ACT, PSUM->SBUF bf16)
  m = min(e-1, 0)                (DVE ts 4x; FRAC_MP share on Pool)
  t = relu(a~) + m  = selu/(s*alpha): ACT Relu + DVE tt-add (in place,
      into the relu tile) for FRAC_TA of tiles, DVE stt (1x, reads PSUM)
      for the rest.  GPSIMD cannot touch PSUM and DMA cannot read PSUM,
      so those are the only two options.
  u = t*t                        (DVE tt 2x, into m's tile; FRAC_UP on Pool)
  q = u_even + u_odd             (Pool tt; pure tail work, HOL-safe)
  sqrt+mean: crude bitcast sqrt on DVE: s ~ bf16(bits(q) >> 1) via a
      single logical-shift ts (walrus rejects mixed bitwise+arith ALU
      pairs and arith shifts); the magic constant's exponent part is an
      exact 2^63 scale folded into CAL, its mantissa part replaced by a
      CAL refit (1.385) -- then per-pair DVE ts-accum (4x) with CAL.
      Rel err ~4e-3 << 2e-2.
Gate: computed in the tail phase (channel sums via row-reduce early + selector
matmul, sigmoid as 1/(1+exp(-x))), overlapping the last chunk's drain.

Schedule: all three compute engines (ACT ~112us, DVE ~112us, Pool
~113us) run ~99%-busy in their windows -- the problem is a genuine
3-engine ridge.  Startup is ~6.5us: tc0 holds ONLY what chunk 0's first
two (K19-path) tiles need, because TileContext exit is a full phase
barrier; lhsT/x_pad/w2/sel load early in main instead.  One warm-up
matmul in tc0 ramps the PE p-state.  The engine-split fractions (FRAC_*)
were tuned against TimelineSim.

_split_multiwait() keeps every instruction at <=1 sync wait (walrus
single-wait codegen limit).

Output rows are stored in (r, chan) x (chunk, p) order and permuted on host.
"""

import os
import numpy as np
import ml_dtypes
from contextlib import ExitStack

import bass_rust
import concourse.bass as bass
import concourse.mybir as mybir
from concourse.tile import TileContext
from concourse.bass_utils import run_bass_kernel_spmd

AF = mybir.ActivationFunctionType
ALU = mybir.AluOpType
AX = mybir.AxisListType
F32 = mybir.dt.float32
BF16 = mybir.dt.bfloat16

SELU_S = 1.0507009873554805
SELU_A = 1.6732632423543772

_CACHE = {}
N_CORES = 8
NPC = 128          # images per core
NPAIR = NPC // 2   # 64 image pairs per core
CH_PAIRS = int(os.environ.get("BASSK_CHP", "16"))  # pairs per im2col chunk
NCHUNK = NPAIR // CH_PAIRS
TPC = CH_PAIRS // 2        # 2-pair tiles per chunk
NTILE = NPAIR // 2         # 2-pair tiles total (32)

FRAC_TA = float(os.environ.get("BASSK_TA", "0.96"))    # t via ACT relu
FRAC_MP = float(os.environ.get("BASSK_MP", "0.125"))    # m on Pool
FRAC_QP = float(os.environ.get("BASSK_QP", "1.0"))    # q on Pool
FRAC_SA = float(os.environ.get("BASSK_SA", "0.0"))    # sqrt groups w/ ACT accum
FRAC_UP = float(os.environ.get("BASSK_UP", "0.25"))    # u on Pool
FRAC_TAL = float(os.environ.get("BASSK_TAL", "0.5"))   # TA for last chunk
CRUDE_SQRT = os.environ.get("BASSK_CRUDE", "1") == "1"
SKIP = set(os.environ.get("BASSK_SKIP", "").split(","))


def _split_multiwait(nc):
    """The walrus build here can only codegen ONE sync-wait per instruction.
    Move extra waits onto prefix no-ops on the same engine (same semantics:
    the sequencer executes the waits in program order before the op)."""
    ctr = 0
    for f in nc.m.functions:
        for blk in f.blocks:
            il = blk.instructions
            i = 0
            while i < len(il):
                ins = il[i]
                si = ins.sync_info
                waits = list(si.on_wait) if (si is not None and si.on_wait) else []
                if len(waits) > 1:
                    for w in waits[:-1]:
                        ctr += 1
                        nop = bass_rust.InstNoOp(name=f"I-mw{ctr}", ins=[], outs=[])
                        nop.engine = ins.engine
                        nop.sync_info = bass_rust.SyncInfo(on_wait=[w], on_update=[])
                        il.insert(i, nop)
                        i += 1
                    ins.sync_info = bass_rust.SyncInfo(
                        on_wait=[waits[-1]], on_update=list(si.on_update or [])
                    )
                i += 1
    return ctr


def _frac_select(idx, total, frac, mode=None):
    """True for ~frac of indices: Bresenham even spread (default),
    'early' = first frac*total indices, 'late' = last ones."""
    if mode is None:
        mode = os.environ.get("BASSK_FSEL", "spread")
    if mode == "early":
        return idx < int(round(total * frac))
    if mode == "late":
        return idx >= total - int(round(total * frac))
    return int((idx + 1) * frac) - int(idx * frac) > 0


def build_nc():
    nc = bass.Bass("TRN2")
    # host-prepped bf16 input layouts: per-chunk 6-row ki=1 base blocks
    # (row (r,c) = x[c, i'-1, j-1] over (i':34, p:16, j:34), pads zeroed)
    # and the padded (c,i)-partitioned x for the gate's channel sums.
    xb_d = nc.dram_tensor("xb_host", (NCHUNK * 6, 34 * CH_PAIRS * 34), BF16,
                          kind="ExternalInput")
    # chunk-0 priority block: all 19 kj=0 rows (ki-expanded + ones) for the
    # first 4 pairs only -- one small DMA unblocks tiles 0/1's K19 matmuls
    xb19p_d = nc.dram_tensor("xb19p_host", (19, 32 * 4 * 34), BF16,
                             kind="ExternalInput")
    xpad_d = nc.dram_tensor("xpad_host", (102, NPC * 34), BF16,
                            kind="ExternalInput")
    # host-packed block-diagonal conv weights (incl. bias ones-row target)
    lhsT_d = nc.dram_tensor("lhsT_host", (57, 128), BF16, kind="ExternalInput")
    lhsT19_d = nc.dram_tensor("lhsT19_host", (19, 3 * 128), BF16, kind="ExternalInput")
    w2_d = nc.dram_tensor("w2_host", (4, 64), BF16, kind="ExternalInput")
    out = nc.dram_tensor("out", (NPC, 64), F32, kind="ExternalOutput")

    ones_dram = nc.inline_tensor(
        np.ones((1, 32 * CH_PAIRS * 34 + 2 * CH_PAIRS * 34), dtype=ml_dtypes.bfloat16), name="ones_row"
    )
    ones_bf = nc.inline_tensor(
        np.ones((1, NPC), dtype=ml_dtypes.bfloat16), name="ones_bf"
    )
    sel_np = np.zeros((102, 4), dtype=np.float32)
    for c in range(3):
        sel_np[34 * c:34 * c + 34, c] = 1.0 / 1024.0
    sel_dram = nc.inline_tensor(sel_np, name="sel_const")

    FREE = 32 * CH_PAIRS * 34  # flat free size of one patch row block

    with ExitStack() as es:
        # persistent SBUF tensors (live across TileContexts)
        x_pad = es.enter_context(nc.sbuf_tensor("x_pad", [102, NPC, 34], BF16))
        lhsT = es.enter_context(nc.sbuf_tensor("lhsT", [57, 128], BF16))
        lhsT19 = es.enter_context(nc.sbuf_tensor("lhsT19", [19, 3 * 128], BF16))
        w2 = es.enter_context(nc.sbuf_tensor("w2", [4, 64], BF16))
        sel = es.enter_context(nc.sbuf_tensor("sel", [102, 4], F32))
        rowsums = es.enter_context(nc.sbuf_tensor("rowsums", [102, NPC], F32))
        csT = es.enter_context(nc.sbuf_tensor("csT", [4, NPC], BF16))
        gexp = es.enter_context(nc.sbuf_tensor("gexp", [128, NPAIR], F32))
        gd = es.enter_context(nc.sbuf_tensor("gd", [128, NPAIR], F32))
        gate = es.enter_context(nc.sbuf_tensor("gate", [128, NPAIR], F32))
        q_all = es.enter_context(nc.sbuf_tensor("q_all", [128, CH_PAIRS * 2 * 512], BF16))
        res = es.enter_context(nc.sbuf_tensor("res", [128, NPAIR], F32))
        # ping-pong im2col patches (persistent so chunk 0's can be built
        # during setup, overlapped with the x load).  Rows carry PAD=544
        # elems of slack each side: the ki=1 rows hold the full 34-i-row
        # transpose (both i-pads) at offset 0, and the ki=0/2 rows are
        # +-544-shifted copies landing in [PAD, PAD+FREE); matmuls read
        # the [PAD, PAD+FREE) window of every row.
        PAD = CH_PAIRS * 34
        FREE2 = FREE + 2 * PAD
        patch_a = es.enter_context(nc.sbuf_tensor("patch_a", [57, FREE2], BF16))
        patch_b = es.enter_context(nc.sbuf_tensor("patch_b", [57, FREE2], BF16))
        patches = [patch_a, patch_b]

        def build_patch(ch, split_queues, dupes=True, swdge_rows=0):
            """im2col for chunk ch into patches[ch % 2]: ONE DRAM DMA
            loads the host-prepped 6-row ki=1 base (kb = 6 + 3r + c) at
            full 34-i-row width; ki=0/2 rows are +-PAD-shifted copies of
            it, and the kj=1/2 blocks are 1/2-elem shifted copies as
            before.  6 HWDGE DMAs/chunk."""
            patch = patches[ch % 2]
            if ch < 2 or os.environ.get("BASSK_ONES", "0") == "1":
                # row 18 is never overwritten afterwards, so chunks 2/3
                # (reusing the same ping-pong buffers) inherit it
                nc.sync.dma_start(out=patch[18:19, :], in_=ones_dram[:, :])
            if 'im2col' not in SKIP:
                nc.sync.dma_start(out=patch[6:12, :],
                                  in_=xb_d[6 * ch:6 * ch + 6, :])
                # ki=0 rows present base[g] at PAD+g; ki=2 present base[2*PAD+g]
                nc.sync.dma_start(out=patch[0:6, PAD:PAD + FREE],
                                  in_=patch[6:12, 0:FREE])
                eng = nc.scalar if split_queues else nc.sync
                eng.dma_start(out=patch[12:18, PAD:PAD + FREE],
                              in_=patch[6:12, 2 * PAD:2 * PAD + FREE])
            if not dupes:
                return patch
            # kj=1 / kj=2 blocks: shifted copies of the base block
            nc.sync.dma_start(out=patch[19:38, 0:FREE2 - 1],
                              in_=patch[0:19, 1:FREE2])
            eng2 = nc.scalar if split_queues else nc.sync
            eng2.dma_start(out=patch[38:57, 0:FREE2 - 2],
                           in_=patch[0:19, 2:FREE2])
            return patch

        # ---- phase 0: setup (weights + chunk 0's patch; all host-prepped
        # bf16, pure HWDGE loads -- the Pool engine does no DMA gen at all)
        with TileContext(nc) as tc0:
            # ONLY what chunk 0's K19 tiles need: TileContext exit is a full
            # phase barrier, so anything else here delays the first matmul.
            # Chunk 0's kj=0 block (incl. ones row) comes pre-expanded from
            # the host as ONE DMA; its first two tiles use the 3-matmul
            # kj-offset scheme on these rows so they need not wait for the
            # kj shift-dupes, issued in main (tiles 2+ use the K=57 path).
            if True:
                p0 = patches[0]
                if os.environ.get("BASSK_P2", "0") == "1":
                    dst19 = p0[0:19, PAD:PAD + FREE].rearrange(
                        "k (i p j) -> k i p j", i=32, p=CH_PAIRS)[:, :, 0:4, :]
                    nc.sync.dma_start(out=dst19, in_=xb19p_d[:, :].rearrange(
                        "k (i p j) -> k i p j", i=32, p=4))
                _sq = os.environ.get("BASSK_SETQ", "mixed")
                for ki in range(3):
                    eng = nc.sync if _sq == "sp" else (
                        nc.scalar if ki == 1 else nc.sync)
                    eng.dma_start(
                        out=p0[6 * ki:6 * ki + 6, PAD:PAD + FREE],
                        in_=xb_d[0:6, PAD * ki:PAD * ki + FREE])
                nc.sync.dma_start(out=p0[18:19, :], in_=ones_dram[:, :])
            # tiny transfer; after the patch loads so its HWDGE gen slot
            # doesn't delay them
            nc.sync.dma_start(out=lhsT19[:, :], in_=lhsT19_d[:, :])
            # PE p-state warm-up: harmless matmuls into a scratch PSUM
            # region keep the PE busy through setup so the first real
            # convolutions run at full clock (ramp needs ~3us of
            # continuous PE activity).
            if 'warm' not in SKIP:
                with nc.psum_tensor("warm_ps", [128, 384], F32) as warm_ps:
                    for _ in range(int(os.environ.get("BASSK_WARM", "1"))):
                        nc.tensor.matmul(warm_ps[:, :], lhsT19[:, 0:128],
                                         lhsT19[:, :], start=True, stop=True)


        with TileContext(nc) as tc:
            with tc.tile_pool(name="workp", bufs=int(os.environ.get("BASSK_WORKBUFS", "3"))) as work_pool, \
                 tc.tile_pool(name="sqp", bufs=int(os.environ.get("BASSK_SQB", "2"))) as sq_pool, \
                 tc.tile_pool(name="psump", bufs=2, space="PSUM") as psum_pool:

                # chunk 0's shift-dupes: issued first thing in main so
                # they overlap tile-0/1 compute (which uses the K=19 path)
                p0 = patches[0]
                nc.sync.dma_start(out=p0[19:28, 0:FREE2 - 1], in_=p0[0:9, 1:FREE2])
                nc.sync.dma_start(out=p0[38:47, 0:FREE2 - 2], in_=p0[0:9, 2:FREE2])
                nc.sync.dma_start(out=p0[28:38, 0:FREE2 - 1], in_=p0[9:19, 1:FREE2])
                nc.sync.dma_start(out=p0[47:57, 0:FREE2 - 2], in_=p0[9:19, 2:FREE2])
                # K=57 weights (tiles 2+), padded x for the gate's channel
                # sums, and tail-phase constants: loaded early in the main
                # loop where HWDGE is idle (in tc0 the phase-exit barrier
                # would delay the first matmul behind them)
                nc.sync.dma_start(out=lhsT[:, :], in_=lhsT_d[:, :])
                nc.sync.dma_start(
                    out=x_pad[:, :, :],
                    in_=xpad_d[:, :].rearrange("c (n j) -> c n j", j=34),
                )
                nc.sync.dma_start(out=w2[:, :], in_=w2_d[:, :])
                nc.sync.dma_start(out=sel[:, :], in_=sel_dram[:, :])
                nc.sync.dma_start(out=csT[3:4, :], in_=ones_bf[:, :])
                # gate inputs: per-(c,i)-row sums (used in the tail phase;
                # scheduled into DVE's wait-for-first-exp window, so its 1x
                # cost is off the critical line)
                nc.vector.tensor_reduce(
                    rowsums[:, :], x_pad[:, :, :], axis=AX.X, op=ALU.add
                )

                # ---- main loop: conv + selu^2 + pair-sum + sqrt-mean ----
                for ch in range(NCHUNK):
                    # Base ki-unfolded patch block (19 rows): partition
                    # k = 9*r + 3*ki + c holds x[c, i+ki-1, row] for the r-th
                    # 8-image half of the chunk; free layout = i*(P*34) +
                    # p*34 + jpad.  Rows 19..37 / 38..56 are the same block
                    # shifted left by 1 / 2 free elements (kj=1 / kj=2), so
                    # ONE K=57 matmul covers all of (c, ki, kj).
                    patch = patches[0] if ch == 0 else build_patch(ch, split_queues=False)

                    pv = patch[:, PAD:PAD + FREE].rearrange(
                        "k (i p j) -> k i p j", i=32, p=CH_PAIRS
                    )
                    for tp in range(TPC):
                        ti = ch * TPC + tp  # global 2-pair tile index
                        gq = 2 * tp        # first pair of tile, within chunk
                        psum_t = psum_pool.tile([128, 2048], F32, tag="conv")
                        for half in (range(2) if 'mm' not in SKIP else []):
                            p = 2 * tp + half
                            if ch == 0 and tp < int(os.environ.get("BASSK_K19T", "2")):
                                # kj via rhs free-offsets + 3 accumulating
                                # matmuls on the base rows (no dupe blocks)
                                for kj in range(3):
                                    lb = lhsT19[:, 128 * kj:128 * kj + 128]
                                    rhs_e = pv[0:19, :, p:p + 1, kj + 0:kj + 31:2]
                                    rhs_o = pv[0:19, :, p:p + 1, kj + 1:kj + 32:2]
                                    nc.tensor.matmul(
                                        psum_t[:, 1024 * half:1024 * half + 512],
                                        lb, rhs_e, start=(kj == 0), stop=(kj == 2))
                                    nc.tensor.matmul(
                                        psum_t[:, 1024 * half + 512:1024 * half + 1024],
                                        lb, rhs_o, start=(kj == 0), stop=(kj == 2))
                            else:
                                rhs_e = pv[:, :, p:p + 1, 0:31:2]
                                rhs_o = pv[:, :, p:p + 1, 1:32:2]
                                nc.tensor.matmul(
                                    psum_t[:, 1024 * half:1024 * half + 512],
                                    lhsT[:, :], rhs_e, start=True, stop=True)
                                nc.tensor.matmul(
                                    psum_t[:, 1024 * half + 512:1024 * half + 1024],
                                    lhsT[:, :], rhs_o, start=True, stop=True)

                        # psum holds a~ = a/alpha (weights pre-scaled on host);
                        # e = exp(alpha*a~) = exp(a); m = min(e-1, 0);
                        # t = relu(a~) + m = selu(a)/(s*alpha).
                        # ACT ops run per 2048-psum-tile; the pure-SBUF ops
                        # (m/t-add/u/q) run on SUPER-tiles of two conv tiles
                        # (4096 free) to halve per-op init/dispatch overhead.
                        # In-place reuse: r2 += m2 turns r2 into t; u is
                        # squared into m2's buffer (m dead after t).
                        if 'elem' in SKIP:
                            continue
                        # super width: last chunk runs per-tile so the drain
                        # chain (exp->m->t->u->q->sqrt->accum) stays short
                        sw = int(os.environ.get("BASSK_SW", "1")) if ch < NCHUNK - 1 else int(os.environ.get("BASSK_SWLAST", "1"))
                        sti = ti // sw
                        nst = NTILE // sw
                        half = tp % sw
                        if half == 0:
                            e2 = work_pool.tile([128, 2048 * sw], BF16, tag="e")
                            st_psums = [None, None]
                        nc.scalar.activation(e2[:, 2048 * half:2048 * half + 2048],
                                             psum_t[:, :], AF.Exp, scale=SELU_A)
                        ta = FRAC_TA if ch < NCHUNK - 1 else FRAC_TAL
                        relu_path = _frac_select(sti, nst, ta)
                        if (ch == NCHUNK - 1 and tp >= TPC - int(os.environ.get("BASSK_LTSTT", "0"))):
                            relu_path = False
                        if relu_path:
                            if half == 0:
                                r2 = work_pool.tile([128, 2048 * sw], BF16, tag="r")
                            nc.scalar.activation(r2[:, 2048 * half:2048 * half + 2048],
                                                 psum_t[:, :], AF.Relu)
                        else:
                            st_psums[half] = psum_t
                        if half < sw - 1:
                            continue
                        # ---- super-tile elementwise (2*sw pairs wide) ----
                        m2 = work_pool.tile([128, 2048 * sw], BF16, tag="m")
                        mp = FRAC_MP if ch < NCHUNK - 1 else float(os.environ.get("BASSK_MPL", "0.1"))
                        meng = nc.gpsimd if _frac_select(sti, nst, mp) else nc.vector
                        meng.tensor_scalar(
                            m2[:, :], e2[:, :], -1.0, 0.0, ALU.add, ALU.min
                        )
                        if relu_path:
                            nc.vector.tensor_tensor(
                                r2[:, :], r2[:, :], m2[:, :], ALU.add
                            )
                            t2 = r2
                        else:
                            t2 = work_pool.tile([128, 2048 * sw], BF16, tag="r")
                            for hh in range(sw):
                                nc.vector.scalar_tensor_tensor(
                                    t2[:, 2048 * hh:2048 * hh + 2048],
                                    st_psums[hh][:, :], 0.0,
                                    m2[:, 2048 * hh:2048 * hh + 2048],
                                    ALU.max, ALU.add,
                                )
                        if 'post_t' in SKIP:
                            continue
                        last_tile = (ch == NCHUNK - 1 and tp == TPC - 1
                                     and os.environ.get("BASSK_LTDVE", "0") == "1")
                        up = FRAC_UP if ch < NCHUNK - 1 else float(os.environ.get("BASSK_UPL", "0.25"))
                        ueng = nc.gpsimd if (_frac_select(sti, nst, up) and not last_tile) else nc.vector
                        ueng.tensor_tensor(
                            m2[:, :], t2[:, :], t2[:, :], ALU.mult
                        )
                        # u = (pair: e 512 | o 512) x 2*sw pairs
                        u4 = m2[:, :].rearrange("z (pr par x) -> z pr par x",
                                                pr=2 * sw, par=2)
                        qp = FRAC_QP if ch < NCHUNK - 1 else float(os.environ.get("BASSK_QPL", "1.0"))
                        qb = (ch % 2) * CH_PAIRS  # double-buffer q_all by chunk
                        gq0 = 2 * (tp - sw + 1)
                        qdst = q_all[:, (qb + gq0) * 512:(qb + gq0 + 2 * sw) * 512]
                        lastq = (ch == NCHUNK - 1 and tp == TPC - 1
                                 and os.environ.get("BASSK_LTQ", "0") == "1")
                        qeng = nc.gpsimd if (_frac_select(sti, nst, qp) and not (last_tile or lastq)) else nc.vector
                        qeng.tensor_tensor(
                            qdst, u4[:, :, 0, :], u4[:, :, 1, :], ALU.add,
                        )

                    # chunk tail: sqrt at FD=4096 (8 pairs/op) on ACT, then
                    # per-pair mean via 4x DVE ts accum_out -- pipelines
                    # behind the next chunk's conv/elementwise.
                    qb = (ch % 2) * CH_PAIRS
                    SCALE = (SELU_S * SELU_A / 512.0) ** 2
                    if CRUDE_SQRT and 'sqrt' not in SKIP:
                        # bitcast sqrt on DVE (4x ts): s ~ bf16(i>>1), shift
                        # ONLY (walrus rejects mixed bitwise+arith ALU ops in
                        # one ts, and arith shifts entirely).  The magic
                        # constant's exponent part (63<<7) is an exact *2^63
                        # scale folded into CAL; its 4 mantissa bits are
                        # replaced by a CAL refit (1.385) for the log-uniform
                        # mantissa distribution -- rel err ~3e-3 << 2e-2.
                        CAL = 1.385 * SELU_S * SELU_A / 512.0 * 2.0 ** 63
                        cw = int(os.environ.get("BASSK_CW", "4")) if ch < NCHUNK - 1 else int(os.environ.get("BASSK_CWL", "4"))  # narrower drain
                        for gg in range(0, CH_PAIRS, cw):
                            qi = q_all[:, (qb + gg) * 512:(qb + gg + cw) * 512]
                            scr = sq_pool.tile([128, 512 * cw], BF16, tag="scr")
                            nc.vector.tensor_scalar(
                                scr[:, :].bitcast(mybir.dt.uint16),
                                qi.bitcast(mybir.dt.uint16),
                                1, None, ALU.logical_shift_right,
                            )
                            for k in range(cw):
                                g = ch * CH_PAIRS + gg + k
                                scr2 = sq_pool.tile([128, 512], BF16, tag="scr2")
                                nc.vector.tensor_scalar(
                                    scr2[:, :], scr[:, k * 512:(k + 1) * 512],
                                    CAL, 0.0, ALU.mult, ALU.add,
                                    accum_out=res[:, g:g + 1],
                                )
                        continue
                    # per 4-pair group: either one Sqrt@2048 + 4 DVE ts-accums,
                    # or 4 individual Sqrt@512 ops with ACT-side accum_out
                    # (shifts the accumulate load DVE -> ACT; accum_out is not
                    # legal on Pool).
                    for gg in (range(0, CH_PAIRS, 4) if 'sqrt' not in SKIP else []):
                        grp = (ch * CH_PAIRS + gg) // 4
                        if _frac_select(grp, NPAIR // 4, FRAC_SA):
                            for k in range(4):
                                g = ch * CH_PAIRS + gg + k
                                scr2 = sq_pool.tile([128, 512], BF16, tag="scr2")
                                nc.scalar.activation(
                                    scr2[:, :],
                                    q_all[:, (qb + gg + k) * 512:(qb + gg + k + 1) * 512],
                                    AF.Sqrt, scale=SCALE,
                                    accum_out=res[:, g:g + 1],
                                )
                        else:
                            scr = sq_pool.tile([128, 2048], BF16, tag="scr")
                            nc.scalar.activation(
                                scr[:, :],
                                q_all[:, (qb + gg) * 512:(qb + gg + 4) * 512],
                                AF.Sqrt, scale=SCALE,
                            )
                            for k in range(4):
                                g = ch * CH_PAIRS + gg + k
                                scr2 = sq_pool.tile([128, 512], BF16, tag="scr2")
                                nc.vector.tensor_scalar(
                                    scr2[:, :], scr[:, k * 512:(k + 1) * 512],
                                    1.0, 0.0, ALU.mult, ALU.add,
                                    accum_out=res[:, g:g + 1],
                                )

                # ---- tail (inside main, overlapping the drain): gate
                # sigmoid path into a borrowed conv-psum tile, multiply, store
                gtile = psum_pool.tile([128, 2048], F32, tag="conv")
                cs_ps = gtile[0:3, 0:NPC]
                nc.tensor.matmul(cs_ps, sel[:, 0:3], rowsums[:, :], start=True, stop=True)
                nc.vector.tensor_copy(csT[0:3, :], cs_ps)

                gp_ps = gtile[:, 256:256 + NPAIR]
                csv = csT[:, :].rearrange("k (ch half p) -> k ch half p", ch=NCHUNK, half=2)
                nc.tensor.matmul(gp_ps[0:64, :], w2[:, :], csv[:, :, 0:1, :], start=True, stop=True)
                nc.tensor.matmul(gp_ps[64:128, :], w2[:, :], csv[:, :, 1:2, :], start=True, stop=True)
                nc.scalar.activation(gexp[:, :], gp_ps, AF.Exp, scale=-1.0)
                nc.vector.tensor_scalar(gd[:, :], gexp[:, :], 1.0, None, ALU.add)
                nc.vector.reciprocal(gate[:, :], gd[:, :])

                outv = sq_pool.tile([128, NPAIR], F32, tag="outv")
                nc.vector.tensor_tensor(outv[:, :], res[:, :], gate[:, :], ALU.mult)
                # identity store; host reorders rows (see kernel())
                nc.sync.dma_start(out=out[:, :], in_=outv[:, :])

    _split_multiwait(nc)
    return nc


def _pack_weights(weight, bias, scale_proj, scale_bias):
    # lhsT rows: 19*kj + 6*ki + 3*r + c ; block-diagonal over the two images
    # of a pair (cols 0:64 / 64:128); row 37 = kj=1 ones-row = conv bias.
    w4 = weight.reshape(3, 3, 3, 64)  # (c, ki, kj, o)
    lhsT = np.zeros((57, 128), dtype=np.float32)
    for kj in range(3):
        for ki in range(3):
            for r in range(2):
                for c in range(3):
                    lhsT[19 * kj + 6 * ki + 3 * r + c, 64 * r:64 * r + 64] = \
                        w4[c, ki, kj, :]
    lhsT[37, 0:64] = bias
    lhsT[37, 64:128] = bias
    lhsT *= 1.0 / SELU_A  # conv emits a/alpha; exp undoes via scale=alpha
    lhsT19 = np.concatenate([lhsT[19 * kj:19 * kj + 19, :] for kj in range(3)],
                            axis=1)  # [19, 3*128], kj blocks along free
    w2 = np.zeros((4, 64), dtype=np.float32)
    w2[0:3] = scale_proj
    w2[3] = scale_bias
    return (lhsT.astype(ml_dtypes.bfloat16), lhsT19.astype(ml_dtypes.bfloat16),
            w2.astype(ml_dtypes.bfloat16))


def kernel(x, weight, bias, scale_proj, scale_bias):
    x = np.ascontiguousarray(np.asarray(x, dtype=np.float32))
    weight = np.asarray(weight, dtype=np.float32)
    bias = np.asarray(bias, dtype=np.float32)
    scale_proj = np.asarray(scale_proj, dtype=np.float32)
    scale_bias = np.asarray(scale_bias, dtype=np.float32)

    lhsT_host, lhsT19_host, w2_host = _pack_weights(weight, bias, scale_proj, scale_bias)

    # host-side layout prep (bf16 cast + zero-padding + im2col base blocks):
    # xp[c, i', n, j'] = padded x; xb[ch, 3r+c, (i' p j')] = the per-chunk
    # ki=1 base rows read by build_patch.
    N = x.shape[0]
    xp = np.zeros((3, 34, N, 34), dtype=ml_dtypes.bfloat16)
    xp[:, 1:33, :, 1:33] = x.transpose(1, 2, 0, 3)
    xpad_full = xp.reshape(102, N * 34)  # [(c,i'), (n,j')] per-core sliced below
    nch_all = N // (2 * CH_PAIRS)
    # xb[ch, 3r+c] = xp[c, :, 32ch+16r : +16, :] flattened (i', p, j')
    xb = np.empty((nch_all, 6, 34 * CH_PAIRS * 34), dtype=ml_dtypes.bfloat16)
    for ch in range(nch_all):
        for r in range(2):
            i0 = 2 * CH_PAIRS * ch + CH_PAIRS * r
            blk = xp[:, :, i0:i0 + CH_PAIRS, :]  # (c, i', p, j)
            xb[ch, 3 * r:3 * r + 3] = blk.reshape(3, -1)

    if "nc" not in _CACHE:
        _CACHE["nc"] = build_nc()
    nc = _CACHE["nc"]

    xpad3 = xpad_full.reshape(102, N // NPC, NPC * 34)
    xb3 = xb.reshape(N // NPC, NCHUNK * 6, -1)
    # chunk-0 priority block per core: 18 ki-expanded rows + ones, pairs 0-3
    xb19ps = []
    for i in range(N // NPC):
        b6 = np.ascontiguousarray(xb3[i][0:6]).reshape(6, 34, CH_PAIRS, 34)
        x19p = np.empty((19, 32 * 4 * 34), dtype=ml_dtypes.bfloat16)
        for ki in range(3):
            for rc in range(6):
                x19p[6 * ki + rc] = b6[rc, ki:ki + 32, 0:4, :].reshape(-1)
        x19p[18] = np.ones((32 * 4 * 34,), dtype=ml_dtypes.bfloat16)
        xb19ps.append(x19p)
    in_maps = [
        {
            "xb_host": np.ascontiguousarray(xb3[i]),
            "xb19p_host": xb19ps[i],
            "xpad_host": np.ascontiguousarray(xpad3[:, i]),
            "lhsT_host": lhsT_host,
            "lhsT19_host": lhsT19_host,
            "w2_host": w2_host,
        }
        for i in range(N_CORES)
    ]
    r = run_bass_kernel_spmd(nc, in_maps, core_ids=list(range(N_CORES)))
    _CACHE["last_result"] = r
    outs = []
    for m in r.results:
        o = m["out"]  # [128 rows = (r, chan), 64 cols = (ch, p)]
        o4 = o.reshape(2, 64, NCHUNK, CH_PAIRS)  # (r, chan, ch, p)
        o4 = o4.transpose(2, 0, 3, 1)            # (ch, r, p, chan)
        outs.append(np.ascontiguousarray(o4.reshape(128, 64)))
    return np.concatenate(outs, axis=0)

